# revision 1
# baseline (speedup 1.0000x reference)
"""Graphormer layer on 8 TRN2 NeuronCores.

Sharding: core c handles batch b = c//2 and query-row half qh = c%2 (1024 q
rows). All compute is in transposed (feature-on-partition) layout; the host
pre-transposes x and the influence slices and transposes per-core outputs
back during the gather. Host also rolls the node axis per core so each core's
own q rows sit at columns [0, 1024) — the device program is identical across
cores (attention over all keys is permutation-invariant; the influence k axis
is rolled identically).

Per core:
  ln1T = LayerNorm_T(xT)              (partition-dim LN via ones-matmul sums)
  QT/KT (f32r matmuls), V natural (bf16)
  per (qc, kc, head-pair):
    psum  = LG[kc]                    (PE identity-copy;  LG = iw1*u + ib1)
    psum += KT_h-slice.T @ QT_h-slice (f32r, contract d=32, row-packed heads)
    e = exp(psum / sqrt(D))           (ACT -> SBUF bf16; includes influence add)
    f = e * G2[kc]                    (DVE bf16;  G2 = iw2*u + ib2)
    wv += V_kc-slice.T @ f            (bf16, col-packed per head)
    Z  += ones.T @ e                  (bf16, col-packed per head)
  attn = (wv / Z) @ Wo + bo ; h = attn + xT_own
  out = W2.T-proj(relu(W1.T-proj(LN_T(h)) + b1)) + b2 + h
"""

import math

import numpy as np

import concourse.bass as bass
import concourse.bacc as bacc
import concourse.mybir as mybir
import concourse.tile as tile
from concourse.bass_utils import run_bass_kernel_spmd

B, N, E, H, D = 4, 2048, 256, 8, 32
NQ = N // 2          # q rows per core
QC = 512             # q window
NKC = N // 128       # 16 k-chunks
EC = E // 128        # 2 feature chunks

f32 = mybir.dt.float32
f32r = mybir.dt.float32r
bf16 = mybir.dt.bfloat16
FT = mybir.ActivationFunctionType
ALU = mybir.AluOpType

# vecs_sb column index: vec v, chunk c -> 2*v + c
V_G1, V_BETA1, V_G2, V_BETA2, V_BO, V_B1, V_B2 = range(7)


def r32(ap):
    return ap.bitcast(f32r)


def layer_norm_T(nc, pp, ps, x_chunks, win, wn, g_col, beta_col, vecs, ones,
                 eps_ap, out_chunks):
    """LayerNorm over the partition dim (E = 2 chunks) in T layout.

    x_chunks: 2 SBUF APs; normalizes cols [win:win+wn] -> out_chunks (f32).
    """
    sq = ps.tile([128, 2 * wn], f32, name="lnsq", tag="lnsq")
    p_s = pp.tile([128, wn], f32, name="lnps", tag="lnps")
    p_sq = pp.tile([128, wn], f32, name="lnpsq", tag="lnpsq")
    for c in range(EC):
        xs = x_chunks[c][:, win:win + wn]
        nc.vector.tensor_mul(sq[:, c * wn:(c + 1) * wn], xs, xs)
        nc.tensor.matmul(p_s[:, :], ones[:, :], xs,
                         start=(c == 0), stop=(c == EC - 1))
    for c in range(EC):
        nc.tensor.matmul(p_sq[:, :], ones[:, :],
                         sq[:, c * wn:(c + 1) * wn],
                         start=(c == 0), stop=(c == EC - 1))
    mu = ps.tile([128, wn], f32, name="lnmu", tag="lnmu")
    msq = ps.tile([128, wn], f32, name="lnmsq", tag="lnmsq")
    nc.vector.tensor_scalar_mul(mu[:, :], p_s[:, :], 1.0 / E)
    nc.vector.tensor_scalar_mul(msq[:, :], p_sq[:, :], 1.0 / E)
    mu2 = ps.tile([128, wn], f32, name="lnmu2", tag="lnmu2")
    nc.vector.tensor_mul(mu2[:, :], mu[:, :], mu[:, :])
    var = ps.tile([128, wn], f32, name="lnvar", tag="lnvar")
    nc.vector.tensor_sub(var[:, :], msq[:, :], mu2[:, :])
    sd = ps.tile([128, wn], f32, name="lnsd", tag="lnsd")
    nc.scalar.activation(sd[:, :], var[:, :], FT.Sqrt, bias=eps_ap)
    rstd = ps.tile([128, wn], f32, name="lnrstd", tag="lnrstd")
    nc.vector.reciprocal_approx_fast(rstd[:, :], sd[:, :])
    for c in range(EC):
        xs = x_chunks[c][:, win:win + wn]
        xm = ps.tile([128, wn], f32, name="lnxm", tag="lnxm")
        nc.vector.tensor_sub(xm[:, :], xs, mu[:, :])
        xm2 = ps.tile([128, wn], f32, name="lnxm2", tag="lnxm2")
        nc.vector.tensor_mul(xm2[:, :], xm[:, :], rstd[:, :])
        nc.vector.tensor_scalar(
            out_chunks[c][:, win:win + wn], xm2[:, :],
            vecs[:, 2 * g_col + c:2 * g_col + c + 1],
            vecs[:, 2 * beta_col + c:2 * beta_col + c + 1],
            ALU.mult, ALU.add)


def build_body(nc, tc, xT_d, inflT_d, w_d, vecs_d, scal_d, ident_d, outT_d):
    persist_pools = []

    def ppool(name):
        p = tc.tile_pool(name=name, bufs=1)
        persist_pools.append(p)
        return p.__enter__()

    persist = ppool("persist")

    # ---- persistent SBUF ----
    qt = [persist.tile([128, NQ], bf16, name=f"qt{c}", tag=f"qt{c}") for c in range(EC)]
    kt = [persist.tile([128, N], bf16, name=f"kt{c}", tag=f"kt{c}") for c in range(EC)]
    xtq = [persist.tile([128, NQ], f32, name=f"xtq{c}", tag=f"xtq{c}") for c in range(EC)]
    v_sb = [persist.tile([128, E], bf16, name=f"v{k}", tag=f"v{k}") for k in range(NKC)]
    ga_sb = [persist.tile([128, NQ], bf16, name=f"ga_{k}", tag=f"ga_{k}") for k in range(NKC)]
    gb_sb = [persist.tile([128, NQ], bf16, name=f"gb_{k}", tag=f"gb_{k}") for k in range(NKC)]
    id_bf = persist.tile([128, 128], bf16, name="id_bf", tag="id_bf")
    w_sb = {n: persist.tile([128, 2 * E], f32, name=f"w_{n}", tag=f"w_{n}") for n in w_d}
    w_bf = {n: persist.tile([128, 2 * E], bf16, name=f"wbf_{n}", tag=f"wbf_{n}")
            for n in w_d}
    vecs = persist.tile([128, 14], f32, name="vecs", tag="vecs")
    scal = persist.tile([128, 4], f32, name="scal", tag="scal")
    ones = persist.tile([128, 128], f32, name="ones", tag="ones")
    ones_bf = persist.tile([128, 32], bf16, name="ones_bf", tag="ones_bf")
    h_sb = [[persist.tile([128, QC], f32, name=f"h{q}{c}", tag=f"h{q}{c}") for c in range(EC)]
            for q in range(2)]

    # ---- small loads ----
    for n in w_d:
        for c in range(EC):
            nc.sync.dma_start(w_sb[n][:, E * c:E * (c + 1)],
                              w_d[n][128 * c:128 * (c + 1), :])
    nc.sync.dma_start(vecs[:, :], vecs_d[:, :])
    nc.sync.dma_start(scal[:, :], scal_d[:, :])
    idt = persist.tile([128, 128], f32, name="id_f32", tag="id_f32")
    nc.sync.dma_start(idt[:, :], ident_d[:, :])
    nc.vector.tensor_copy(id_bf[:, :], idt[:, :])
    eps_t = persist.tile([128, 1], f32, name="eps_t", tag="eps_t")
    nc.vector.memset(eps_t[:, :], 1e-5)
    nc.vector.memset(ones[:, :], 1.0)
    nc.vector.memset(ones_bf[:, :], 1.0)
    for n in w_d:
        nc.vector.tensor_copy(w_bf[n][:, :], w_sb[n][:, :])

    # ---- stage B/C: LN1 + projections (xt/ln1 are stage-local) ----
    with tc.tile_pool(name="xt_pool", bufs=1) as xp, \
         tc.tile_pool(name="ln_psum", bufs=2, space="PSUM") as ln_pp, \
         tc.tile_pool(name="ln_sbuf", bufs=2) as ln_ps, \
         tc.tile_pool(name="proj_psum", bufs=2, space="PSUM") as proj_psum:
        xt = [xp.tile([128, N], f32, name=f"xt{c}", tag=f"xt{c}") for c in range(EC)]
        ln1 = [xp.tile([128, N], bf16, name=f"ln1{c}", tag=f"ln1{c}") for c in range(EC)]
        for c in range(EC):
            nc.sync.dma_start(xt[c][:, :], xT_d[128 * c:128 * (c + 1), :])
            nc.vector.tensor_copy(xtq[c][:, :], xt[c][:, :NQ])
        for w in range(N // 512):
            layer_norm_T(nc, ln_pp, ln_ps, xt, 512 * w, 512, V_G1, V_BETA1,
                         vecs, ones, eps_t[:, :], ln1)
        for fc in range(EC):
            for qw in range(NQ // 512):
                pq = proj_psum.tile([128, 512], f32, name="proj", tag="proj")
                for ec in range(EC):
                    nc.tensor.matmul(
                        pq[:, :],
                        w_bf["Wq"][:, E * ec + 128 * fc:E * ec + 128 * (fc + 1)],
                        ln1[ec][:, 512 * qw:512 * (qw + 1)],
                        start=(ec == 0), stop=(ec == EC - 1))
                nc.vector.tensor_copy(qt[fc][:, 512 * qw:512 * (qw + 1)], pq[:, :])
        for fc in range(EC):
            for kw in range(N // 512):
                pk = proj_psum.tile([128, 512], f32, name="proj", tag="proj")
                for ec in range(EC):
                    nc.tensor.matmul(
                        pk[:, :],
                        w_bf["Wk"][:, E * ec + 128 * fc:E * ec + 128 * (fc + 1)],
                        ln1[ec][:, 512 * kw:512 * (kw + 1)],
                        start=(ec == 0), stop=(ec == EC - 1))
                nc.vector.tensor_copy(kt[fc][:, 512 * kw:512 * (kw + 1)], pk[:, :])
        for k in range(NKC):
            pv = proj_psum.tile([128, E], f32, name="projv", tag="projv")
            for ec in range(EC):
                nc.tensor.matmul(
                    pv[:, :],
                    ln1[ec][:, 128 * k:128 * (k + 1)],
                    w_bf["Wv"][:, E * ec:E * (ec + 1)],
                    start=(ec == 0), stop=(ec == EC - 1))
            nc.vector.tensor_copy(v_sb[k][:, :], pv[:, :])

    # ---- stage D (hybrid): even kc -> LG,G2 ; odd kc -> EG,G3 (bf16) ----
    with tc.tile_pool(name="gprep", bufs=3) as gp:
        for k in range(NKC):
            u = gp.tile([128, NQ], f32, name="u", tag="u")
            nc.sync.dma_start(u[:, :], inflT_d[128 * k:128 * (k + 1), :])
            if k % 2 == 0:
                nc.vector.tensor_scalar(ga_sb[k][:, :], u[:, :], scal[:, 0:1],
                                        scal[:, 1:2], ALU.mult, ALU.add)
                nc.vector.tensor_scalar(gb_sb[k][:, :], u[:, :], scal[:, 2:3],
                                        scal[:, 3:4], ALU.mult, ALU.add)
            else:
                nc.scalar.activation(ga_sb[k][:, :], u[:, :], FT.Exp,
                                     scale=scal[:, 0:1], bias=scal[:, 1:2])
                g2t = gp.tile([128, NQ], bf16, name="g2t", tag="g2t")
                nc.vector.tensor_scalar(g2t[:, :], u[:, :], scal[:, 2:3],
                                        scal[:, 3:4], ALU.mult, ALU.add)
                nc.vector.tensor_mul(gb_sb[k][:, :], ga_sb[k][:, :], g2t[:, :])

    # ---- stage E: attention ----
    inv_sqrt_d = 1.0 / math.sqrt(D)
    with tc.tile_pool(name="score_psum", bufs=2, space="PSUM") as sp, \
         tc.tile_pool(name="acc_psum", bufs=1, space="PSUM") as ap_, \
         tc.tile_pool(name="ef_sbuf", bufs=6) as efp, \
         tc.tile_pool(name="att_sbuf", bufs=2) as asb:
        for qc in range(2):
            q0 = QC * qc
            wv_ps = [ap_.tile([128, QC], f32, name=f"wv{s}", tag=f"wv{s}") for s in range(2)]
            z_ps = [ap_.tile([128, QC], f32, name=f"z{s}", tag=f"z{s}") for s in range(2)]
            for kc in range(NKC):
                for half in range(2):  # head sets {0-3}, {4-7}
                    sts = []
                    for hg in (2 * half, 2 * half + 1):
                        st = sp.tile([128, 2 * QC], f32, name="score", tag="score")
                        sts.append((st, hg))
                    even = (kc % 2 == 0)
                    if even:
                        # LG preloads (full-array identity copies, keeps PE hot)
                        for st, hg in sts:
                            for j in range(2):
                                nc.tensor.matmul(
                                    st[:, QC * j:QC * (j + 1)],
                                    id_bf[:, :],
                                    ga_sb[kc][:, q0:q0 + QC],
                                    start=True, stop=False)
                    # 4 q.k matmuls back-to-back on distinct row-groups -> pack
                    for st, hg in sts:
                        for j in range(2):
                            h = 2 * hg + j
                            c, hh = h // 4, 32 * (h % 4)
                            nc.tensor.matmul(
                                st[:, QC * j:QC * (j + 1)],
                                kt[c][hh:hh + 32, 128 * kc:128 * (kc + 1)],
                                qt[c][hh:hh + 32, q0:q0 + QC],
                                start=not even, stop=True,
                                skip_group_check=True, tile_position=(hh, 0))
                    gab = ga_sb[kc][:, q0:q0 + QC].rearrange(
                        "p (o q) -> p o q", o=1).broadcast_to([128, 2, QC])
                    gbb = gb_sb[kc][:, q0:q0 + QC].rearrange(
                        "p (o q) -> p o q", o=1).broadcast_to([128, 2, QC])
                    for st, hg in sts:
                        e = efp.tile([128, 2 * QC], bf16, name="e", tag="e")
                        nc.scalar.activation(e[:, :], st[:, :], FT.Exp)
                        er = e[:, :].rearrange("p (o q) -> p o q", o=2)
                        if even:
                            zsrc = e    # e already includes the influence add
                        else:
                            zsrc = efp.tile([128, 2 * QC], bf16, name="t", tag="t")
                            nc.vector.tensor_tensor(
                                zsrc[:, :].rearrange("p (o q) -> p o q", o=2),
                                er, gab, ALU.mult)
                        for j in range(2):
                            h = 2 * hg + j
                            s_, hh = h // 4, 32 * (h % 4)
                            nc.tensor.matmul(
                                z_ps[s_][hh:hh + 32, :],
                                ones_bf[:, :],
                                zsrc[:, QC * j:QC * (j + 1)],
                                start=(kc == 0), stop=(kc == NKC - 1),
                                skip_group_check=True, tile_position=(0, hh))
                        f = efp.tile([128, 2 * QC], bf16, name="f", tag="f")
                        nc.vector.tensor_tensor(
                            f[:, :].rearrange("p (o q) -> p o q", o=2),
                            er, gbb, ALU.mult)
                        for j in range(2):
                            h = 2 * hg + j
                            s_, hh = h // 4, 32 * (h % 4)
                            nc.tensor.matmul(
                                wv_ps[s_][hh:hh + 32, :],
                                v_sb[kc][:, 32 * h:32 * h + 32],
                                f[:, QC * j:QC * (j + 1)],
                                start=(kc == 0), stop=(kc == NKC - 1),
                                skip_group_check=True, tile_position=(0, hh))
            # normalize + Wo projection + bias + residual -> h
            on = []
            for s in range(2):
                zr = asb.tile([128, QC], f32, name=f"zr{s}", tag=f"zr{s}")
                nc.vector.reciprocal_approx_fast(zr[:, :], z_ps[s][:, :])
                o = asb.tile([128, QC], bf16, name=f"on{s}", tag=f"on{s}")
                nc.vector.tensor_mul(o[:, :], wv_ps[s][:, :], zr[:, :])
                on.append(o)
            for fc in range(EC):
                po = sp.tile([128, QC], f32, name="score", tag="score")
                for ec in range(EC):
                    nc.tensor.matmul(
                        po[:, :],
                        w_bf["Wo"][:, E * ec + 128 * fc:E * ec + 128 * (fc + 1)],
                        on[ec][:, :],
                        start=(ec == 0), stop=(ec == EC - 1))
                ta = asb.tile([128, QC], f32, name="tattn", tag="tattn")
                nc.vector.tensor_scalar_add(ta[:, :], po[:, :],
                                            vecs[:, 2 * V_BO + fc:2 * V_BO + fc + 1])
                nc.vector.tensor_add(h_sb[qc][fc][:, :], ta[:, :],
                                     xtq[fc][:, q0:q0 + QC])

    # ---- stage F: LN2 + FFN + residual + store ----
    with tc.tile_pool(name="ln_psum2", bufs=2, space="PSUM") as ln_pp2, \
         tc.tile_pool(name="ln_sbuf2", bufs=2) as ln_ps2, \
         tc.tile_pool(name="ffn_psum", bufs=2, space="PSUM") as fp, \
         tc.tile_pool(name="ffn_sbuf", bufs=2) as fs:
        for qc in range(2):
            ln2 = [fs.tile([128, QC], bf16, name=f"ln2{c}", tag=f"ln2{c}") for c in range(EC)]
            layer_norm_T(nc, ln_pp2, ln_ps2, h_sb[qc], 0, QC, V_G2, V_BETA2,
                         vecs, ones, eps_t[:, :], ln2)
            z1 = [fs.tile([128, QC], bf16, name=f"z1{c}", tag=f"z1{c}") for c in range(EC)]
            for fc in range(EC):
                p1 = fp.tile([128, QC], f32, name="ffn", tag="ffn")
                for ec in range(EC):
                    nc.tensor.matmul(
                        p1[:, :],
                        w_bf["W1"][:, E * ec + 128 * fc:E * ec + 128 * (fc + 1)],
                        ln2[ec][:, :],
                        start=(ec == 0), stop=(ec == EC - 1))
                nc.vector.tensor_scalar(z1[fc][:, :], p1[:, :],
                                        vecs[:, 2 * V_B1 + fc:2 * V_B1 + fc + 1],
                                        0.0, ALU.add, ALU.max)
            for fc in range(EC):
                p2 = fp.tile([128, QC], f32, name="ffn", tag="ffn")
                for ec in range(EC):
                    nc.tensor.matmul(
                        p2[:, :],
                        w_bf["W2"][:, E * ec + 128 * fc:E * ec + 128 * (fc + 1)],
                        z1[ec][:, :],
                        start=(ec == 0), stop=(ec == EC - 1))
                t2 = fs.tile([128, QC], f32, name="t2", tag="t2")
                nc.vector.tensor_scalar_add(t2[:, :], p2[:, :],
                                            vecs[:, 2 * V_B2 + fc:2 * V_B2 + fc + 1])
                of = fs.tile([128, QC], f32, name="of", tag="of")
                nc.vector.tensor_add(of[:, :], t2[:, :], h_sb[qc][fc][:, :])
                nc.sync.dma_start(
                    outT_d[128 * fc:128 * (fc + 1), QC * qc:QC * (qc + 1)],
                    of[:, :])

    for p in reversed(persist_pools):
        p.__exit__(None, None, None)


def build_nc():
    nc = bacc.Bacc(
        "TRN2",
        target_bir_lowering=False,
        debug=False,
        enable_asserts=False,
        num_devices=8,
    )
    xT_d = nc.dram_tensor("xT", [E, N], f32, kind="ExternalInput").ap()
    inflT_d = nc.dram_tensor("inflT", [N, NQ], f32, kind="ExternalInput").ap()
    w_d = {
        name: nc.dram_tensor(name, [E, E], f32, kind="ExternalInput").ap()
        for name in ("Wq", "Wk", "Wv", "Wo", "W1", "W2")
    }
    vecs_d = nc.dram_tensor("vecs", [128, 14], f32, kind="ExternalInput").ap()
    scal_d = nc.dram_tensor("scal", [128, 4], f32, kind="ExternalInput").ap()
    ident_d = nc.dram_tensor("ident", [128, 128], f32, kind="ExternalInput").ap()
    outT_d = nc.dram_tensor("outT", [E, NQ], f32, kind="ExternalOutput").ap()

    with tile.TileContext(nc) as tc:
        build_body(nc, tc, xT_d, inflT_d, w_d, vecs_d, scal_d, ident_d, outT_d)
    nc.compile()
    return nc


def host_shard(inputs):
    """Build the 8 per-core input maps (see module docstring for the roll)."""
    x = np.asarray(inputs["x"], np.float32)
    infl = np.asarray(inputs["influence_matrix"], np.float32)
    vec_list = ["g1", "beta1", "g2", "beta2", "bo", "b1", "b2"]
    vecs_np = np.empty((128, 14), np.float32)
    for vi, nm in enumerate(vec_list):
        v = np.asarray(inputs[nm], np.float32).reshape(E)
        vecs_np[:, 2 * vi] = v[:128]
        vecs_np[:, 2 * vi + 1] = v[128:]
    scal_np = np.tile(
        np.array([inputs["iw1"], inputs["ib1"], inputs["iw2"], inputs["ib2"]],
                 np.float32).reshape(1, 4), (128, 1))
    # Fold the 1/sqrt(D) score scale into Q host-side? No: fold into Wq here.
    ws = {n: np.ascontiguousarray(np.asarray(inputs[n], np.float32))
          for n in ("Wq", "Wk", "Wv", "Wo", "W1", "W2")}
    ws["Wq"] = ws["Wq"] / math.sqrt(D)

    in_maps = []
    for core in range(8):
        b, qh = core // 2, core % 2
        qoff = qh * NQ
        xb = np.roll(x[b], -qoff, axis=0)          # [N, E], own rows first
        xT = np.ascontiguousarray(xb.T)            # [E, N]
        inf_slice = np.roll(infl[b][qoff:qoff + NQ, :], -qoff, axis=1)
        inflT = np.ascontiguousarray(inf_slice.T)  # [N(k), NQ]
        m = {"xT": xT, "inflT": inflT, "vecs": vecs_np, "scal": scal_np,
             "ident": np.eye(128, dtype=np.float32)}
        m.update(ws)
        in_maps.append(m)
    return in_maps


_NC_CACHE = []


def kernel(**inputs):
    if not _NC_CACHE:
        _NC_CACHE.append(build_nc())
    nc = _NC_CACHE[0]
    in_maps = host_shard(inputs)
    res = run_bass_kernel_spmd(nc, in_maps, core_ids=list(range(8)))
    out = np.empty((B, N, E), np.float32)
    for core in range(8):
        b, qh = core // 2, core % 2
        out[b, qh * NQ:(qh + 1) * NQ, :] = np.asarray(
            res.results[core]["outT"], np.float32).T
    return out

